# revision 11
# baseline (speedup 1.0000x reference)
"""Trainium2 Bass kernel for nn_Decoder_17214228922493.

32-step LSTM decoder: B=64, H=1536, input=1024, applied to a constant input.
    xg = x @ W_ih.T + b_ih + b_hh
    per step: gates = xg + h @ W_hh.T ; LSTM cell update ; emit h

Sharding: tensor-parallel over the gate dimension (8 cores x 768 gate
columns); after every step the 8 h^T slices are re-assembled with an
AllGather (mesh, ~5us).

Matmul layout: PE column-tiling.  The stationary operand (h^T k-tile,
M=64) is loaded twice — into array column-groups 0-1 (tile_position
(0,0)) and 2-3 ((0,64)) — so two 192-column W streams run concurrently
and the gates land in ONE [128, 384] PSUM bank: partitions 0-63 = gate
cols [f|i], partitions 64-127 = [g|o].

Layout tricks (all index permutations are host-side, the PE only needs
h^T k-tiles and W_hh k-tiles to agree on contraction order):
 - dense reload: h-dim d(k,p) = 12p + k, so the AllGather output
   [1536, 64] is read back as one [128, 768] DMA whose partition lines
   are contiguous 1536B runs (vs 1536 strided 128B descriptors).
 - even/odd bounce: core-local slot j holds global dim 192r + sigma(j),
   sigma = evens then odds, so the packed transpose tile's line p
   carries dims (2p, 2p+1) = one contiguous 256B DRAM run; the bounce
   is 96 descriptors instead of 192.
 - weights are packed host-side into [128, k*cols] blobs so startup is
   a handful of full-line DMAs.

Each phase gets its own PSUM bank ({f,g} then {i,o}); phase-0 eltwise
overlaps the phase-1 matmuls.  The serial tail after the last matmul is
sigmoid(i)/(o) -> i*g -> c -> tanh(c) -> h -> 2 PE transposes -> one
bounce DMA.

The x-matmul contribution (constant over steps) is re-computed every
step during the AllGather window, followed by filler matmuls into a
scratch bank: the PE queue runs h-matmuls -> transposes -> x-burst ->
fillers -> (next) h-matmuls with no idle window >3.4us, so the HAM
clock never throttles to 1.2GHz.

Outputs accumulate in SBUF ([64, 32*192]) and are written with one DMA
at the end, keeping the gpsimd queue free so the collective doorbell
fires as soon as the bounce lands.
"""

import sys

if "/opt/trn_rl_repo" not in sys.path:
    sys.path.insert(0, "/opt/trn_rl_repo")

from contextlib import ExitStack

import ml_dtypes
import numpy as np

import concourse.bass as bass
import concourse.mybir as mybir
import concourse.tile as tile
from concourse import bacc
from concourse import bass_utils
from concourse._compat import get_trn_type

F32 = mybir.dt.float32
BF16 = mybir.dt.bfloat16
R = 8          # cores
B = 64         # batch
H = 1536       # hidden
HL = H // R    # 192 per-core hidden slice
IN = 1024      # lstm input size
KA = 1152      # augmented input contraction (1024 + bias row, padded to 9*128)
NG = 4 * HL    # 768 gate columns per core
S = 32         # steps
PH = 192       # phase width (gate cols per phase per partition-half)
KHT = H // 128   # 12 k-tiles for the recurrent matmul
KAT = KA // 128  # 9 k-tiles for the input matmul
NFILL = 48     # filler matmuls ([64,384] each, ~0.16us) bridging PE to reload

_CACHE = {}


def _build():
    nc = bacc.Bacc(
        get_trn_type() or "TRN2",
        target_bir_lowering=False,
        debug=False,
        num_devices=R,
    )

    xw = nc.dram_tensor("xw", [128, KAT * B], BF16, kind="ExternalInput")
    wih = nc.dram_tensor("wih", [128, KAT * NG], BF16, kind="ExternalInput")
    whh = nc.dram_tensor("whh", [128, KHT * NG], BF16, kind="ExternalInput")
    h0T = nc.dram_tensor("h0T", [H, B], BF16, kind="ExternalInput")
    c0 = nc.dram_tensor("c0", [B, HL], F32, kind="ExternalInput")
    iden = nc.dram_tensor("iden", [B, B], BF16, kind="ExternalInput")
    out = nc.dram_tensor("out", [B, S * HL], BF16, kind="ExternalOutput")

    bounces = [
        nc.dram_tensor(f"bounce{t}", [HL, B], BF16, kind="Internal") for t in range(S)
    ]
    gaths = [
        nc.dram_tensor(f"gath{t}", [H, B], BF16, kind="Internal", addr_space="Shared")
        for t in range(S - 1)
    ]
    warm_in = nc.dram_tensor("warm_in", [1, B], BF16, kind="Internal")
    warm_out = nc.dram_tensor("warm_out", [R, B], BF16, kind="Internal",
                              addr_space="Shared")

    sig = mybir.ActivationFunctionType.Sigmoid
    tanh = mybir.ActivationFunctionType.Tanh

    # rhs column windows: (partition-half, phase) -> W col slice within a
    # k-tile's NG-wide block
    #   half A (p0-63):  cols 0:192 = f, 192:384 = i
    #   half B (p64-127): cols 384:576 = g, 576:768 = o
    def wslice(k, half, phase):
        base = k * NG + half * 2 * PH + phase * PH
        return slice(base, base + PH)

    with ExitStack() as ctx:
        tc = ctx.enter_context(tile.TileContext(nc))
        wpool = ctx.enter_context(tc.tile_pool(name="w", bufs=1))
        cpool = ctx.enter_context(tc.tile_pool(name="cst", bufs=1))
        hpool = ctx.enter_context(tc.tile_pool(name="h", bufs=2))
        spool = ctx.enter_context(tc.tile_pool(name="s", bufs=3))
        opool = ctx.enter_context(tc.tile_pool(name="o", bufs=1))
        gpool = ctx.enter_context(tc.tile_pool(name="g", bufs=2, space="PSUM"))
        tpool = ctx.enter_context(tc.tile_pool(name="t", bufs=1, space="PSUM"))

        # dummy collective fired first: pays the ~40us first-call ncfw cost
        # during the startup window instead of on step 0's critical path
        nc.gpsimd.collective_compute(
            "AllGather",
            mybir.AluOpType.bypass,
            replica_groups=[list(range(R))],
            ins=[warm_in[:]],
            outs=[warm_out[:]],
        )

        # packed startup loads, one full-line DMA each, spread across queues
        xw_t = wpool.tile([128, KAT * B], BF16, tag="xw")
        nc.sync.dma_start(xw_t[:], xw[:])
        wih_t = wpool.tile([128, KAT * NG], BF16, tag="wih")
        nc.scalar.dma_start(wih_t[:], wih[:])
        whh_t = wpool.tile([128, KHT * NG], BF16, tag="whh")
        nc.sync.dma_start(whh_t[:], whh[:])
        iden_t = cpool.tile([B, B], BF16, tag="iden")
        nc.gpsimd.dma_start(iden_t[:], iden[:])
        c_t = spool.tile([B, HL], F32, tag="c")
        nc.gpsimd.dma_start(c_t[:], c0[:])
        outbig = opool.tile([B, S * HL], BF16, tag="outbig")

        # dense h^T reload: partition p line = gather rows 12p..12p+11
        # (contiguous 1536B), i.e. h-dim d(k,p) = 12p + k; split in two
        # contiguous chunks on two queues so both issue in parallel.
        def load_hT(src, engines):
            src2d = src.rearrange("(p k) n -> p (k n)", p=128)
            hs = []
            for i, eng in enumerate(engines):
                hc = hpool.tile([128, 4 * B], BF16, tag=f"hh{i}")
                eng.dma_start(hc[:], src2d[:, 4 * i * B : 4 * (i + 1) * B])
                hs.append(hc)
            return hs

        def h_tile(hs, k):
            return hs[k // 4][:, (k % 4) * B : (k % 4 + 1) * B]

        h_halves = load_hT(h0T, (nc.gpsimd, nc.scalar, nc.gpsimd))

        for t in range(S):
            # one PSUM bank per phase: ps[0] = {f @ p0-63, g @ p64-127},
            # ps[1] = {i @ p0-63, o @ p64-127}.  start=True clears the
            # has_written bits of the written PARTITIONS' whole bank rows, so
            # each (bank, partition-half) gets exactly one start — the k==0
            # x-matmul — and nothing else may clear it afterwards.
            ps0 = gpool.tile([128, PH], F32, tag="g0")
            ps1 = gpool.tile([128, PH], F32, tag="g1")
            ps = [ps0, ps1]

            # x-burst: 36 matmuls with no h dependency, gated on the previous
            # bounce DMA so it runs inside the AllGather window and does NOT
            # overlap the eltwise tail (PE SBUF/PSUM port contention slows
            # the tail ops ~1.7x if it does).
            prev_k0 = None
            xlast = None
            for phase in range(2):
                for half in range(2):
                    outap = ps[phase][64 * half : 64 * (half + 1), :]
                    for k in range(KAT):
                        mm = nc.tensor.matmul(
                            outap,
                            xw_t[:, k * B : (k + 1) * B],
                            wih_t[:, wslice(k, half, phase)],
                            start=(k == 0),
                            stop=False,
                            skip_group_check=True,
                        )
                        if k == 0:
                            if prev_k0 is None and t > 0:
                                tile.add_dep_helper(
                                    mm.ins,
                                    bounce_dma.ins,
                                    sync=True,
                                    reason="x-burst starts after the bounce",
                                )
                            if prev_k0 is not None:
                                tile.add_dep_helper(
                                    mm.ins,
                                    prev_k0.ins,
                                    sync=False,
                                    reason="quadrant queue order",
                                )
                            prev_k0 = mm
                        xlast = mm
            # filler matmuls into a scratch bank bridge the PE from burst-end
            # to reload-land so the HAM clock never sees an idle >3.4us window
            scr = gpool.tile([B, 2 * PH], F32, tag="scr")
            prev_fill = xlast
            for _ in range(0 if t == 0 else NFILL):
                fm = nc.tensor.matmul(
                    scr[:], xw_t[:, 0:B], wih_t[:, 0 : 2 * PH],
                    start=True, stop=True, skip_group_check=True,
                )
                tile.add_dep_helper(
                    fm.ins, prev_fill.ins, sync=False, reason="filler chain"
                )
                prev_fill = fm
            # h-matmuls, phase-major so phase 0's gates finish first
            xlast = prev_fill
            first_h = True
            for phase in range(2):
                for k in range(KHT):
                    outap_h = [
                        ps[phase][0:64, :],
                        ps[phase][64:128, :],
                    ]
                    for half in range(2):
                        mm = nc.tensor.matmul(
                            outap_h[half],
                            h_tile(h_halves, k),
                            whh_t[:, wslice(k, half, phase)],
                            start=False,
                            stop=(k == KHT - 1),
                            skip_group_check=True,
                        )
                        if first_h:
                            first_h = False
                            tile.add_dep_helper(
                                mm.ins,
                                xlast.ins,
                                sync=False,
                                reason="x-burst precedes h-matmuls in PE queue",
                            )

            # phase-0 eltwise overlaps phase-1 matmuls
            sf = spool.tile([B, PH], F32, tag="sf")
            nc.scalar.activation(sf[:], ps[0][0:64, :], sig)
            tg = spool.tile([B, PH], F32, tag="tg")
            nc.scalar.activation(tg[:], ps[0][64:128, :], tanh)
            m1 = spool.tile([B, PH], F32, tag="m1")
            nc.vector.tensor_mul(m1[:], sf[:], c_t[:])  # f*c

            # phase-1 tail
            si = spool.tile([B, PH], F32, tag="si")
            nc.scalar.activation(si[:], ps[1][0:64, :], sig)
            so = spool.tile([B, PH], BF16, tag="so")
            nc.scalar.activation(so[:], ps[1][64:128, :], sig)
            m2 = spool.tile([B, PH], F32, tag="m2")
            nc.vector.tensor_mul(m2[:], si[:], tg[:])  # i*g
            c_new = spool.tile([B, HL], F32, tag="c")
            nc.vector.tensor_add(c_new[:], m1[:], m2[:])
            c_t = c_new
            tc_sb = spool.tile([B, PH], BF16, tag="tc")
            hm = outbig[:, t * HL : (t + 1) * HL]
            tp = tpool.tile([96, 2 * B], BF16, tag="tp")
            for hh in range(2):  # column halves pipeline into one packed tile
                csl = slice(96 * hh, 96 * (hh + 1))
                nc.scalar.activation(tc_sb[:, csl], c_new[:, csl], tanh)
                nc.vector.tensor_mul(hm[:, csl], so[:, csl], tc_sb[:, csl])
                nc.tensor.transpose(tp[:, B * hh : B * (hh + 1)], hm[:, csl], iden_t[:])
            tsb = spool.tile([96, 2 * B], BF16, tag="tsb")
            nc.vector.tensor_copy(tsb[:], tp[:])
            # bounce: tsb line p = local slots (p, 96+p) = global chunk rows
            # (2p, 2p+1) under sigma -> one contiguous 256B run per line
            bounce_dma = nc.sync.dma_start(
                bounces[t].rearrange("(p b) n -> p b n", p=96),
                tsb.rearrange("p (b n) -> p b n", b=2),
            )

            if t < S - 1:
                nc.gpsimd.collective_compute(
                    "AllGather",
                    mybir.AluOpType.bypass,
                    replica_groups=[list(range(R))],
                    ins=[bounces[t][:]],
                    outs=[gaths[t][:]],
                )
                h_halves = load_hT(gaths[t], (nc.sync, nc.scalar, nc.gpsimd))

        # single output write at the end (64 lines x 12KB)
        nc.gpsimd.dma_start(out[:], outbig[:])

    nc.compile()
    return nc


# even/odd permutation: local slot j <-> chunk row sigma(j)
SIGMA = np.concatenate([2 * np.arange(96), 2 * np.arange(96) + 1])


def _prep_inputs(sequence, hidden_state, cell_state, W_ih, W_hh, b_ih, b_hh):
    x = np.asarray(sequence, np.float32)[0]          # [64, 1024]
    h0 = np.asarray(hidden_state, np.float32)[0]     # [64, 1536]
    c0f = np.asarray(cell_state, np.float32)[0]
    W_ih = np.asarray(W_ih, np.float32)
    W_hh = np.asarray(W_hh, np.float32)
    b = (np.asarray(b_ih, np.float32) + np.asarray(b_hh, np.float32))

    bf = ml_dtypes.bfloat16
    xT = np.zeros((KA, B), np.float32)
    xT[:IN] = x.T
    xT[IN] = 1.0
    # pack to [128, KAT*B]: partition p, k-tile k -> row 128k + p
    xw = np.ascontiguousarray(
        xT.reshape(KAT, 128, B).transpose(1, 0, 2).reshape(128, KAT * B)
    ).astype(bf)
    h0T = np.ascontiguousarray(h0.T).astype(bf)
    iden = np.eye(B, dtype=bf)
    # dense-reload permutation: whh k-tile k, partition p holds h-dim 12p+k
    hperm = (12 * np.arange(128)[:, None] + np.arange(KHT)[None, :]).reshape(-1)

    in_maps = []
    for r in range(R):
        sl = r * HL + SIGMA
        # per-core gate column order: f, i, g, o
        sel = np.concatenate([H + sl, sl, 2 * H + sl, 3 * H + sl])
        wa = np.zeros((KA, NG), np.float32)
        wa[:IN] = W_ih[sel].T
        wa[IN] = b[sel]
        wih_p = np.ascontiguousarray(
            wa.reshape(KAT, 128, NG).transpose(1, 0, 2).reshape(128, KAT * NG)
        ).astype(bf)
        whhT = np.ascontiguousarray(W_hh[sel].T)       # [1536, 768]
        whh_p = np.ascontiguousarray(
            whhT[hperm].reshape(128, KHT * NG)
        ).astype(bf)
        in_maps.append(
            {
                "xw": xw,
                "wih": wih_p,
                "whh": whh_p,
                "h0T": h0T,
                "c0": np.ascontiguousarray(c0f[:, sl]),
                "iden": iden,
            }
        )
    return in_maps


def kernel(**inputs) -> np.ndarray:
    if "nc" not in _CACHE:
        _CACHE["nc"] = _build()
    nc = _CACHE["nc"]
    in_maps = _prep_inputs(**inputs)
    res = bass_utils.run_bass_kernel_spmd(nc, in_maps, core_ids=list(range(R)))
    preds = np.empty((S, B, H), np.float32)
    for r in range(R):
        o = np.asarray(res.results[r]["out"], np.float32)  # [64, 32*192]
        preds[:, :, r * HL + SIGMA] = o.reshape(B, S, HL).transpose(1, 0, 2)
    return preds


# revision 18
# speedup vs baseline: 1.1380x; 1.1380x over previous
"""Trainium2 Bass kernel for nn_Decoder_17214228922493.

32-step LSTM decoder: B=64, H=1536, input=1024, applied to a constant input.
    xg = x @ W_ih.T + b_ih + b_hh
    per step: gates = xg + h @ W_hh.T ; LSTM cell update ; emit h

Sharding: tensor-parallel over the gate dimension (8 cores x 768 gate
columns); after every step the 8 h^T slices are re-assembled with an
AllGather (mesh, ~5us).

Matmul layout: PE column-tiling.  The stationary operand (h^T k-tile,
M=64) is loaded twice — into array column-groups 0-1 (tile_position
(0,0)) and 2-3 ((0,64)) — so two 192-column W streams run concurrently
and the gates land in ONE [128, 384] PSUM bank: partitions 0-63 = gate
cols [f|i], partitions 64-127 = [g|o].

Layout tricks (all index permutations are host-side, the PE only needs
h^T k-tiles and W_hh k-tiles to agree on contraction order):
 - dense reload: h-dim d(k,p) = 12p + k, so the AllGather output
   [1536, 64] is read back as one [128, 768] DMA whose partition lines
   are contiguous 1536B runs (vs 1536 strided 128B descriptors).
 - even/odd bounce: core-local slot j holds global dim 192r + sigma(j),
   sigma = evens then odds, so the packed transpose tile's line p
   carries dims (2p, 2p+1) = one contiguous 256B DRAM run; the bounce
   is 96 descriptors instead of 192.
 - weights are packed host-side into [128, k*cols] blobs so startup is
   a handful of full-line DMAs.

Each phase gets its own PSUM bank ({f,g} then {i,o}); phase-0 eltwise
overlaps the phase-1 matmuls.  The serial tail after the last matmul is
sigmoid(i)/(o) -> i*g -> c -> tanh(c) -> h -> 2 PE transposes -> one
bounce DMA.

The x-matmul contribution (constant over steps) is re-computed every
step during the AllGather window, followed by filler matmuls into a
scratch bank: the PE queue runs h-matmuls -> transposes -> x-burst ->
fillers -> (next) h-matmuls with no idle window >3.4us, so the HAM
clock never throttles to 1.2GHz.

Outputs accumulate in SBUF ([64, 32*192]) and are written with one DMA
at the end, keeping the gpsimd queue free so the collective doorbell
fires as soon as the bounce lands.
"""

import sys

if "/opt/trn_rl_repo" not in sys.path:
    sys.path.insert(0, "/opt/trn_rl_repo")

from contextlib import ExitStack

import ml_dtypes
import numpy as np

import concourse.bass as bass
import concourse.mybir as mybir
import concourse.tile as tile
from concourse import bacc
from concourse import bass_utils
from concourse._compat import get_trn_type

F32 = mybir.dt.float32
BF16 = mybir.dt.bfloat16
R = 8          # cores
B = 64         # batch
H = 1536       # hidden
HL = H // R    # 192 per-core hidden slice
IN = 1024      # lstm input size
KA = 1152      # augmented input contraction (1024 + bias row, padded to 9*128)
NG = 4 * HL    # 768 gate columns per core
S = 32         # steps
PH = 192       # phase width (gate cols per phase per partition-half)
KHT = H // 128   # 12 k-tiles for the recurrent matmul
KAT = KA // 128  # 9 k-tiles for the input matmul
NFILL = 30     # filler matmuls ([64,384] each, ~0.2us) bridging PE to reload

_CACHE = {}


def _build():
    nc = bacc.Bacc(
        get_trn_type() or "TRN2",
        target_bir_lowering=False,
        debug=False,
        num_devices=R,
    )

    xw = nc.dram_tensor("xw", [128, KAT * B], BF16, kind="ExternalInput")
    wih = nc.dram_tensor("wih", [128, KAT * NG], BF16, kind="ExternalInput")
    whh = nc.dram_tensor("whh", [128, KHT * NG], BF16, kind="ExternalInput")
    h0T = nc.dram_tensor("h0T", [H, B], BF16, kind="ExternalInput")
    c0 = nc.dram_tensor("c0", [B, HL], F32, kind="ExternalInput")
    iden = nc.dram_tensor("iden", [B, B], BF16, kind="ExternalInput")
    out = nc.dram_tensor("out", [B, S * HL], BF16, kind="ExternalOutput")

    bounces = [
        nc.dram_tensor(f"bounce{t}", [HL, B], BF16, kind="Internal") for t in range(S)
    ]
    gaths = [
        nc.dram_tensor(f"gath{t}", [H, B], BF16, kind="Internal", addr_space="Shared")
        for t in range(S - 1)
    ]


    sig = mybir.ActivationFunctionType.Sigmoid
    tanh = mybir.ActivationFunctionType.Tanh

    # rhs column windows: (partition-half, phase) -> W col slice within a
    # k-tile's NG-wide block
    #   half A (p0-63):  cols 0:192 = f, 192:384 = i
    #   half B (p64-127): cols 384:576 = g, 576:768 = o
    def wslice(k, half, phase):
        base = k * NG + half * 2 * PH + phase * PH
        return slice(base, base + PH)

    with ExitStack() as ctx:
        tc = ctx.enter_context(tile.TileContext(nc))
        wpool = ctx.enter_context(tc.tile_pool(name="w", bufs=1))
        cpool = ctx.enter_context(tc.tile_pool(name="cst", bufs=1))
        hpool = ctx.enter_context(tc.tile_pool(name="h", bufs=2))
        spool = ctx.enter_context(tc.tile_pool(name="s", bufs=3))
        opool = ctx.enter_context(tc.tile_pool(name="o", bufs=1))
        gpool = ctx.enter_context(tc.tile_pool(name="g", bufs=2, space="PSUM"))
        tpool = ctx.enter_context(tc.tile_pool(name="t", bufs=1, space="PSUM"))

        # packed startup loads, one full-line DMA each, spread across queues
        xw_t = wpool.tile([128, KAT * B], BF16, tag="xw")
        nc.sync.dma_start(xw_t[:], xw[:])
        wih_t = wpool.tile([128, KAT * NG], BF16, tag="wih")
        nc.scalar.dma_start(wih_t[:], wih[:])
        whh_t = wpool.tile([128, KHT * NG], BF16, tag="whh")
        nc.sync.dma_start(whh_t[:], whh[:])
        iden_t = cpool.tile([B, B], BF16, tag="iden")
        nc.gpsimd.dma_start(iden_t[:], iden[:])
        c_t = spool.tile([B, HL], F32, tag="c")
        nc.gpsimd.dma_start(c_t[:], c0[:])
        outbig = opool.tile([B, S * HL], BF16, tag="outbig")

        # warm-work scratch: the scalar and vector engines clock-throttle
        # after a few us of idle (ops run ~1.6x slow when cold), so chained
        # dummy ops keep them at full clock through the AllGather window.
        wsa = cpool.tile([B, 768], F32, tag="wsa")
        wsb = cpool.tile([B, 768], F32, tag="wsb")
        wva = cpool.tile([B, 1536], F32, tag="wva")
        wvb = cpool.tile([B, 1536], F32, tag="wvb")

        # dense h^T reload: partition p line = gather rows 12p..12p+11
        # (contiguous 1536B), i.e. h-dim d(k,p) = 12p + k; split in two
        # contiguous chunks on two queues so both issue in parallel.
        def load_hT(src, engines):
            src2d = src.rearrange("(p k) n -> p (k n)", p=128)
            hs = []
            for i, eng in enumerate(engines):
                hc = hpool.tile([128, 4 * B], BF16, tag=f"hh{i}")
                eng.dma_start(hc[:], src2d[:, 4 * i * B : 4 * (i + 1) * B])
                hs.append(hc)
            return hs

        def h_tile(hs, k):
            return hs[k // 4][:, (k % 4) * B : (k % 4 + 1) * B]

        h_halves = load_hT(h0T, (nc.gpsimd, nc.scalar, nc.gpsimd))

        for t in range(S):
            # one PSUM bank per phase: ps[0] = {f @ p0-63, g @ p64-127},
            # ps[1] = {i @ p0-63, o @ p64-127}.  start=True clears the
            # has_written bits of the written PARTITIONS' whole bank rows, so
            # each (bank, partition-half) gets exactly one start — the k==0
            # x-matmul — and nothing else may clear it afterwards.
            ps0 = gpool.tile([128, PH], F32, tag="g0")
            ps1 = gpool.tile([128, PH], F32, tag="g1")
            ps = [ps0, ps1]

            # x-burst: 36 matmuls with no h dependency, gated on the previous
            # bounce DMA so it runs inside the AllGather window and does NOT
            # overlap the eltwise tail (PE SBUF/PSUM port contention slows
            # the tail ops ~1.7x if it does).
            prev_k0 = None
            xlast = None
            for phase in range(2):
                for half in range(2):
                    outap = ps[phase][64 * half : 64 * (half + 1), :]
                    for k in range(KAT):
                        mm = nc.tensor.matmul(
                            outap,
                            xw_t[:, k * B : (k + 1) * B],
                            wih_t[:, wslice(k, half, phase)],
                            start=(k == 0),
                            stop=False,
                            skip_group_check=True,
                        )
                        if k == 0:
                            if prev_k0 is None and t > 0:
                                tile.add_dep_helper(
                                    mm.ins,
                                    bounce_dma.ins,
                                    sync=True,
                                    reason="x-burst starts after the bounce",
                                )
                            if prev_k0 is not None:
                                tile.add_dep_helper(
                                    mm.ins,
                                    prev_k0.ins,
                                    sync=False,
                                    reason="quadrant queue order",
                                )
                            prev_k0 = mm
                        xlast = mm
            # filler matmuls into a scratch bank bridge the PE from burst-end
            # to reload-land so the HAM clock never sees an idle >3.4us window
            scr = gpool.tile([B, 2 * PH], F32, tag="scr")
            prev_fill = xlast
            for _ in range(0 if t == 0 else NFILL):
                fm = nc.tensor.matmul(
                    scr[:], xw_t[:, 0:B], wih_t[:, 0 : 2 * PH],
                    start=True, stop=True, skip_group_check=True,
                )
                tile.add_dep_helper(
                    fm.ins, prev_fill.ins, sync=False, reason="filler chain"
                )
                prev_fill = fm
            # h-matmuls, phase-major so phase 0's gates finish first
            xlast = prev_fill
            first_h = True
            for phase in range(2):
                for k in range(KHT):
                    outap_h = [
                        ps[phase][0:64, :],
                        ps[phase][64:128, :],
                    ]
                    for half in range(2):
                        mm = nc.tensor.matmul(
                            outap_h[half],
                            h_tile(h_halves, k),
                            whh_t[:, wslice(k, half, phase)],
                            start=False,
                            stop=(k == KHT - 1),
                            skip_group_check=True,
                        )
                        if first_h:
                            first_h = False
                            tile.add_dep_helper(
                                mm.ins,
                                xlast.ins,
                                sync=False,
                                reason="x-burst precedes h-matmuls in PE queue",
                            )

            # phase-0 eltwise overlaps phase-1 matmuls
            sf = spool.tile([B, PH], F32, tag="sf")
            nc.scalar.activation(sf[:], ps[0][0:64, :], sig)
            tg = spool.tile([B, PH], F32, tag="tg")
            nc.scalar.activation(tg[:], ps[0][64:128, :], tanh)
            m1 = spool.tile([B, PH], F32, tag="m1")
            nc.vector.tensor_mul(m1[:], sf[:], c_t[:])  # f*c
            # bridge work so the engines stay warm across the phase-1 wait
            nc.scalar.activation(wsa[:], wsb[:], sig)
            nc.scalar.activation(wsb[:], wsa[:], sig)
            nc.vector.tensor_copy(wva[:, 0:768], wvb[:, 0:768])

            # phase-1 tail
            si = spool.tile([B, PH], F32, tag="si")
            nc.scalar.activation(si[:], ps[1][0:64, :], sig)
            so = spool.tile([B, PH], BF16, tag="so")
            nc.scalar.activation(so[:], ps[1][64:128, :], sig)
            m2 = spool.tile([B, PH], F32, tag="m2")
            nc.vector.tensor_mul(m2[:], si[:], tg[:])  # i*g
            c_new = spool.tile([B, HL], F32, tag="c")
            nc.vector.tensor_add(c_new[:], m1[:], m2[:])
            c_t = c_new
            tc_sb = spool.tile([B, PH], BF16, tag="tc")
            hm = outbig[:, t * HL : (t + 1) * HL]
            tp = tpool.tile([96, 2 * B], BF16, tag="tp")
            for hh in range(2):  # column halves pipeline into one packed tile
                csl = slice(96 * hh, 96 * (hh + 1))
                nc.scalar.activation(tc_sb[:, csl], c_new[:, csl], tanh)
                nc.vector.tensor_mul(hm[:, csl], so[:, csl], tc_sb[:, csl])
                nc.tensor.transpose(tp[:, B * hh : B * (hh + 1)], hm[:, csl], iden_t[:])
            tsb = spool.tile([96, 2 * B], BF16, tag="tsb")
            nc.vector.tensor_copy(tsb[:], tp[:])
            # bounce: tsb line p = local slots (p, 96+p) = global chunk rows
            # (2p, 2p+1) under sigma -> one contiguous 256B run per line
            bounce_dma = nc.sync.dma_start(
                bounces[t].rearrange("(p b) n -> p b n", p=96),
                tsb.rearrange("p (b n) -> p b n", b=2),
            )

            if t < S - 1:
                # warm chains through the AllGather window, anchored on the
                # bounce; emitted before load_hT so the scalar chain runs
                # ahead of its (blocking) reload issue in queue order
                for i in range(7):
                    sa = nc.scalar.activation(
                        (wsa if i % 2 == 0 else wsb)[:],
                        (wsb if i % 2 == 0 else wsa)[:], sig)
                    va = nc.vector.tensor_copy(
                        (wva if i % 2 == 0 else wvb)[:],
                        (wvb if i % 2 == 0 else wva)[:])
                    if i == 0:
                        tile.add_dep_helper(
                            sa.ins, bounce_dma.ins, sync=True,
                            reason="scalar warm chain starts at bounce")
                        tile.add_dep_helper(
                            va.ins, bounce_dma.ins, sync=True,
                            reason="vector warm chain starts at bounce")
                nc.gpsimd.collective_compute(
                    "AllGather",
                    mybir.AluOpType.bypass,
                    replica_groups=[list(range(R))],
                    ins=[bounces[t][:]],
                    outs=[gaths[t][:]],
                )
                h_halves = load_hT(gaths[t], (nc.sync, nc.scalar, nc.gpsimd))

        # single output write at the end (64 lines x 12KB)
        nc.gpsimd.dma_start(out[:], outbig[:])

    nc.compile()
    return nc


# even/odd permutation: local slot j <-> chunk row sigma(j)
SIGMA = np.concatenate([2 * np.arange(96), 2 * np.arange(96) + 1])


def _prep_inputs(sequence, hidden_state, cell_state, W_ih, W_hh, b_ih, b_hh):
    x = np.asarray(sequence, np.float32)[0]          # [64, 1024]
    h0 = np.asarray(hidden_state, np.float32)[0]     # [64, 1536]
    c0f = np.asarray(cell_state, np.float32)[0]
    W_ih = np.asarray(W_ih, np.float32)
    W_hh = np.asarray(W_hh, np.float32)
    b = (np.asarray(b_ih, np.float32) + np.asarray(b_hh, np.float32))

    bf = ml_dtypes.bfloat16
    xT = np.zeros((KA, B), np.float32)
    xT[:IN] = x.T
    xT[IN] = 1.0
    # pack to [128, KAT*B]: partition p, k-tile k -> row 128k + p
    xw = np.ascontiguousarray(
        xT.reshape(KAT, 128, B).transpose(1, 0, 2).reshape(128, KAT * B)
    ).astype(bf)
    h0T = np.ascontiguousarray(h0.T).astype(bf)
    iden = np.eye(B, dtype=bf)
    # dense-reload permutation: whh k-tile k, partition p holds h-dim 12p+k
    hperm = (12 * np.arange(128)[:, None] + np.arange(KHT)[None, :]).reshape(-1)

    in_maps = []
    for r in range(R):
        sl = r * HL + SIGMA
        # per-core gate column order: f, i, g, o
        sel = np.concatenate([H + sl, sl, 2 * H + sl, 3 * H + sl])
        wa = np.zeros((KA, NG), np.float32)
        wa[:IN] = W_ih[sel].T
        wa[IN] = b[sel]
        wih_p = np.ascontiguousarray(
            wa.reshape(KAT, 128, NG).transpose(1, 0, 2).reshape(128, KAT * NG)
        ).astype(bf)
        whhT = np.ascontiguousarray(W_hh[sel].T)       # [1536, 768]
        whh_p = np.ascontiguousarray(
            whhT[hperm].reshape(128, KHT * NG)
        ).astype(bf)
        in_maps.append(
            {
                "xw": xw,
                "wih": wih_p,
                "whh": whh_p,
                "h0T": h0T,
                "c0": np.ascontiguousarray(c0f[:, sl]),
                "iden": iden,
            }
        )
    return in_maps


def kernel(**inputs) -> np.ndarray:
    if "nc" not in _CACHE:
        _CACHE["nc"] = _build()
    nc = _CACHE["nc"]
    in_maps = _prep_inputs(**inputs)
    res = bass_utils.run_bass_kernel_spmd(nc, in_maps, core_ids=list(range(R)))
    preds = np.empty((S, B, H), np.float32)
    for r in range(R):
        o = np.asarray(res.results[r]["out"], np.float32)  # [64, 32*192]
        preds[:, :, r * HL + SIGMA] = o.reshape(B, S, HL).transpose(1, 0, 2)
    return preds
